# revision 4
# baseline (speedup 1.0000x reference)
"""Trainium2 Bass kernel for nn_BandProcessor (temporal decay window +
neighbor window + FFN transformer block).

Contract: kernel(**inputs) takes the FULL inputs as numpy arrays and
returns the FULL [8, 8192, 256] float32 output, distributing batch
elements across the 8 NeuronCores (pure data parallel; DxD params
replicated).

Self-contained: hardcodes all shapes; no sibling imports.
"""

import numpy as np
import ml_dtypes

import concourse.bacc as bacc
import concourse.mybir as mybir
from concourse.tile import TileContext
from concourse import bass_utils

B, T, D = 8, 8192, 256
H = 16
DECAY = 0.9
EPS = 1e-5
NT = T // 128          # 64 token tiles per core
SB = 16                # tiles per superblock (FFN batching)
NSB = NT // SB         # 4 superblocks

F32 = mybir.dt.float32
F32R = mybir.dt.float32r
BF16 = mybir.dt.bfloat16

AF = mybir.ActivationFunctionType
ALU = mybir.AluOpType


# ---------------------------------------------------------------- host prep

def _host_consts(inp):
    """Fold LN gains + value/out projections into single matrices (f64)."""
    g1, b1_ = inp["n1_g"].astype(np.float64), inp["n1_b"].astype(np.float64)
    g2, b2_ = inp["n2_g"].astype(np.float64), inp["n2_b"].astype(np.float64)
    g3, b3_ = inp["n3_g"].astype(np.float64), inp["n3_b"].astype(np.float64)
    t_Wv, t_bv = inp["t_Wv"].astype(np.float64), inp["t_bv"].astype(np.float64)
    t_Wo, t_bo = inp["t_Wo"].astype(np.float64), inp["t_bo"].astype(np.float64)
    a_Wv, a_bv = inp["a_Wv"].astype(np.float64), inp["a_bv"].astype(np.float64)
    a_Wo, a_bo = inp["a_Wo"].astype(np.float64), inp["a_bo"].astype(np.float64)
    f_W1, f_b1 = inp["f_W1"].astype(np.float64), inp["f_b1"].astype(np.float64)
    f_W2, f_b2 = inp["f_W2"].astype(np.float64), inp["f_b2"].astype(np.float64)

    WtWo = t_Wv @ t_Wo
    WaWo = a_Wv @ a_Wo
    Wt_eff = (g1[:, None] * WtWo).astype(np.float32)          # [D, D]
    bt_eff = (b1_ @ WtWo + t_bv @ t_Wo + t_bo).astype(np.float32)
    Wa_eff = (g2[:, None] * WaWo).astype(np.float32)
    ba_eff = (b2_ @ WaWo + a_bv @ a_Wo + a_bo).astype(np.float32)
    W1_eff = (g3[:, None] * f_W1).astype(np.float32)          # [D, 2D]
    b1_eff = (b3_ @ f_W1 + f_b1).astype(np.float32)           # [2D]
    W2 = f_W2.astype(np.float32)                              # [2D, D]
    b2 = f_b2.astype(np.float32)

    # temporal weights: reference weights tw[j] apply to h_pad[j:j+T], i.e.
    # lag delta = H-1-j  ->  w_lag[delta] = tw[H-1-delta]
    tw = DECAY ** np.arange(H, dtype=np.float64)
    tw = tw / tw.sum()
    w_lag = tw[::-1].copy()   # w_lag[d] weights h[t-d]

    band1c = np.zeros((128, 128), np.float64)
    for ti in range(128):
        for to in range(ti, min(128, ti + H)):
            band1c[ti, to] = w_lag[to - ti]
    band1p = np.zeros((128, 128), np.float64)
    for p in range(113, 128):
        for to in range(0, p - 112):          # lag = to + 128 - p in [1, 15]
            band1p[p, to] = w_lag[to + 128 - p]
    band2c = np.zeros((128, 128), np.float64)
    for ti in range(128):
        for to in range(max(0, ti - 1), min(128, ti + 2)):
            band2c[ti, to] = 1.0 / 3.0
    e_prev = np.zeros((128, 128), np.float64); e_prev[127, 0] = 1.0 / 3.0
    e_prev0 = np.zeros((128, 128), np.float64); e_prev0[0, 0] = 1.0 / 3.0
    e_next = np.zeros((128, 128), np.float64); e_next[0, 127] = 1.0 / 3.0
    e_next63 = np.zeros((128, 128), np.float64); e_next63[127, 127] = 1.0 / 3.0

    bf = lambda a: a.astype(ml_dtypes.bfloat16)

    # first-tile correction for the temporal zero-pad of the LN bias term
    c_t = np.cumsum(w_lag)[:H - 1]            # c(t) for t = 0..14
    corr = ((c_t - 1.0)[:, None] * (b1_ @ WtWo)[None, :]).astype(np.float32)

    consts = {
        "wt": np.stack([Wt_eff[0:128], Wt_eff[128:256]]),      # [2,128,256]
        "wa": np.stack([Wa_eff[0:128], Wa_eff[128:256]]),
        "w1": np.stack([W1_eff[0:128], W1_eff[128:256]]),      # [2,128,512]
        "w2": np.stack([W2[k * 128:(k + 1) * 128] for k in range(4)]),  # [4,128,256]
        "band1c": bf(band1c), "band1p": bf(band1p),
        "band2c": bf(band2c), "e_prev": bf(e_prev), "e_prev0": bf(e_prev0),
        "e_next": bf(e_next), "e_next63": bf(e_next63),
        "ones_r": np.ones((1, 128), np.float32),
        "brow": np.stack([bt_eff, ba_eff, b2]),                # [3,256]
        "b1col": b1_eff.reshape(4, 128).T.copy(),              # [128,4]
        "ident": np.eye(128, dtype=np.float32),
    }
    need_corr = bool(np.abs(corr).max() > 0)
    return consts, corr, need_corr


# ---------------------------------------------------------------- bass build

def build_nc(repeat=1, need_corr=False):
    nc = bacc.Bacc("TRN2", target_bir_lowering=False, debug=False, num_devices=8)

    x_d = nc.dram_tensor("x", (T, D), F32, kind="ExternalInput")
    out_d = nc.dram_tensor("out", (T, D), F32, kind="ExternalOutput")
    wt_d = nc.dram_tensor("wt", (2, 128, 256), F32R, kind="ExternalInput")
    wa_d = nc.dram_tensor("wa", (2, 128, 256), F32R, kind="ExternalInput")
    w1_d = nc.dram_tensor("w1", (2, 128, 512), F32R, kind="ExternalInput")
    w2_d = nc.dram_tensor("w2", (4, 128, 256), F32R, kind="ExternalInput")
    bands_d = {}
    for nm in ("band1c", "band1p", "band2c", "e_prev", "e_prev0", "e_next", "e_next63"):
        bands_d[nm] = nc.dram_tensor(nm, (128, 128), BF16, kind="ExternalInput")
    ones_d = nc.dram_tensor("ones_r", (1, 128), F32R, kind="ExternalInput")
    brow_d = nc.dram_tensor("brow", (3, 256), F32R, kind="ExternalInput")
    b1c_d = nc.dram_tensor("b1col", (128, 4), F32, kind="ExternalInput")
    id_d = nc.dram_tensor("ident", (128, 128), F32, kind="ExternalInput")
    corr_d = nc.dram_tensor("corr", (15, 256), F32, kind="ExternalInput") if need_corr else None

    with TileContext(nc) as tc:
        import contextlib
        ctx = contextlib.ExitStack()
        with ctx:
            consts = ctx.enter_context(tc.tile_pool(name="consts", bufs=1))
            xpool = ctx.enter_context(tc.tile_pool(name="xpool", bufs=3))
            xn1p = ctx.enter_context(tc.tile_pool(name="xn1p", bufs=3))
            x1p = ctx.enter_context(tc.tile_pool(name="x1p", bufs=20))
            xn2p = ctx.enter_context(tc.tile_pool(name="xn2p", bufs=20))
            x2p = ctx.enter_context(tc.tile_pool(name="x2p", bufs=34))
            xn3p = ctx.enter_context(tc.tile_pool(name="xn3p", bufs=3))
            aggsb = ctx.enter_context(tc.tile_pool(name="aggsb", bufs=3))
            bigp = ctx.enter_context(tc.tile_pool(name="bigp", bufs=2))
            gelup = ctx.enter_context(tc.tile_pool(name="gelup", bufs=1))
            outp = ctx.enter_context(tc.tile_pool(name="outp", bufs=3))
            smalls = ctx.enter_context(tc.tile_pool(name="smalls", bufs=8))
            aggps = ctx.enter_context(tc.tile_pool(name="aggps", bufs=2, space="PSUM"))
            attps = ctx.enter_context(tc.tile_pool(name="attps", bufs=2, space="PSUM"))
            gargps = ctx.enter_context(tc.tile_pool(name="gargps", bufs=1, space="PSUM"))

            # ---- load constants once
            wt_sb = consts.tile([128, 2, 256], F32R)
            wa_sb = consts.tile([128, 2, 256], F32R)
            w1_sb = consts.tile([128, 2, 512], F32R)
            w2_sb = consts.tile([128, 4, 256], F32R)
            for k in range(2):
                nc.sync.dma_start(out=wt_sb[:, k, :], in_=wt_d[k, :, :])
                nc.sync.dma_start(out=wa_sb[:, k, :], in_=wa_d[k, :, :])
                nc.sync.dma_start(out=w1_sb[:, k, :], in_=w1_d[k, :, :])
            for k in range(4):
                nc.sync.dma_start(out=w2_sb[:, k, :], in_=w2_d[k, :, :])
            band_sb = {}
            for nm, d in bands_d.items():
                tb = consts.tile([128, 128], BF16, tag=nm)
                nc.sync.dma_start(out=tb, in_=d[:, :])
                band_sb[nm] = tb
            ones_sb = consts.tile([1, 128], F32R, tag="ones")
            nc.sync.dma_start(out=ones_sb, in_=ones_d[:, :])
            brow_sb = consts.tile([1, 3, 256], F32R, tag="brow")
            nc.sync.dma_start(out=brow_sb, in_=brow_d[:, :])
            b1_sb = consts.tile([128, 4], F32, tag="b1c")
            nc.sync.dma_start(out=b1_sb, in_=b1c_d[:, :])
            id_sb = consts.tile([128, 128], F32, tag="ident")
            nc.sync.dma_start(out=id_sb, in_=id_d[:, :])
            eps_sb = consts.tile([128, 1], F32, tag="eps")
            nc.vector.memset(eps_sb, EPS)
            corr_sb = None
            if need_corr:
                corr_sb = consts.tile([15, 256], F32, tag="corr")
                nc.sync.dma_start(out=corr_sb, in_=corr_d[:, :])

            # per-iteration state (tile handles keyed by tile index)
            st = {}

            def ln(src_ap, out_tile):
                s = smalls.tile([128, 6], F32, tag="bnst")
                nc.vector.bn_stats(s, src_ap)
                mv = smalls.tile([128, 2], F32, tag="bnmv")
                nc.vector.bn_aggr(mv, s)
                std = smalls.tile([128, 1], F32, tag="std")
                nc.scalar.activation(std, mv[:, 1:2], AF.Sqrt, bias=eps_sb[:, 0:1])
                rstd = smalls.tile([128, 1], F32, tag="rstd")
                nc.vector.reciprocal(rstd, std)
                nc.vector.tensor_scalar(out=out_tile, in0=src_ap,
                                        scalar1=mv[:, 0:1], scalar2=rstd,
                                        op0=ALU.subtract, op1=ALU.mult)

            def front(i):
                xt = xpool.tile([128, 256], F32, tag="x")
                nc.sync.dma_start(out=xt, in_=x_d[i * 128:(i + 1) * 128, :])
                xn1 = xn1p.tile([128, 256], BF16, tag="xn1")
                ln(xt, xn1)
                st[("xn1", i)] = xn1
                # temporal conv -> agg1^T (C layout) in psum
                a1ps = aggps.tile([128, 2, 128], F32, tag="aggT")
                for h in range(2):
                    hs = slice(h * 128, (h + 1) * 128)
                    nc.tensor.matmul(a1ps[:, h, :], xn1[:, hs], band_sb["band1c"],
                                     start=True, stop=(i == 0))
                    if i > 0:
                        nc.tensor.matmul(a1ps[:, h, :], st[("xn1", i - 1)][:, hs],
                                         band_sb["band1p"], start=False, stop=True)
                a1sb = aggsb.tile([128, 2, 128], F32R, tag="aggTsb")
                nc.scalar.activation(a1sb, a1ps, AF.Copy)
                att = attps.tile([128, 256], F32, tag="att")
                nc.tensor.matmul(att, a1sb[:, 0, :], wt_sb[:, 0, :], start=True, stop=False)
                nc.tensor.matmul(att, a1sb[:, 1, :], wt_sb[:, 1, :], start=False, stop=False)
                nc.tensor.matmul(att, ones_sb, brow_sb[:, 0, :], start=False, stop=True)
                x1 = x1p.tile([128, 256], F32, tag="x1")
                nc.vector.tensor_add(out=x1, in0=xt, in1=att)
                if need_corr and i == 0:
                    nc.vector.tensor_add(out=x1[0:15, :], in0=x1[0:15, :], in1=corr_sb)
                st[("x1", i)] = x1
                xn2 = xn2p.tile([128, 256], BF16, tag="xn2")
                ln(x1, xn2)
                st[("xn2", i)] = xn2

            def mid(j):
                xn2 = st[("xn2", j)]
                xn2_prev = st[("xn2", j - 1)] if j > 0 else st[("xn2", 0)]
                xn2_next = st[("xn2", j + 1)] if j < NT - 1 else st[("xn2", NT - 1)]
                eprev = band_sb["e_prev"] if j > 0 else band_sb["e_prev0"]
                enext = band_sb["e_next"] if j < NT - 1 else band_sb["e_next63"]
                a2ps = aggps.tile([128, 2, 128], F32, tag="aggT")
                for h in range(2):
                    hs = slice(h * 128, (h + 1) * 128)
                    nc.tensor.matmul(a2ps[:, h, :], xn2[:, hs], band_sb["band2c"],
                                     start=True, stop=False)
                    nc.tensor.matmul(a2ps[:, h, :], xn2_prev[:, hs], eprev,
                                     start=False, stop=False)
                    nc.tensor.matmul(a2ps[:, h, :], xn2_next[:, hs], enext,
                                     start=False, stop=True)
                a2sb = aggsb.tile([128, 2, 128], F32R, tag="aggTsb")
                nc.scalar.activation(a2sb, a2ps, AF.Copy)
                att = attps.tile([128, 256], F32, tag="att")
                nc.tensor.matmul(att, a2sb[:, 0, :], wa_sb[:, 0, :], start=True, stop=False)
                nc.tensor.matmul(att, a2sb[:, 1, :], wa_sb[:, 1, :], start=False, stop=False)
                nc.tensor.matmul(att, ones_sb, brow_sb[:, 1, :], start=False, stop=True)
                x2 = x2p.tile([128, 256], F32, tag="x2")
                nc.vector.tensor_add(out=x2, in0=st[("x1", j)], in1=att)
                st[("x2", j)] = x2
                xn3 = xn3p.tile([128, 256], F32, tag="xn3")
                ln(x2, xn3)
                # transpose xn3 -> C layout, append into superblock buffer
                x3ps = aggps.tile([128, 2, 128], F32, tag="aggT")
                for h in range(2):
                    nc.tensor.transpose(x3ps[:, h, :], xn3[:, h * 128:(h + 1) * 128], id_sb)
                buf = st[("xn3T", j // SB)]
                jr = j % SB
                nc.scalar.activation(buf[:, :, jr * 128:(jr + 1) * 128], x3ps, AF.Copy)

            def ffn(sb):
                xbuf = st[("xn3T", sb)]
                gbuf = gelup.tile([128, 4, 4 * 512], F32R, tag="geluT")
                for q in range(4):
                    qs = slice(q * 512, (q + 1) * 512)
                    gps = gargps.tile([128, 4, 512], F32, tag="garg")
                    for m in range(4):
                        ms = slice(m * 128, (m + 1) * 128)
                        nc.tensor.matmul(gps[:, m, :], w1_sb[:, 0, ms], xbuf[:, 0, qs],
                                         start=True, stop=False)
                        nc.tensor.matmul(gps[:, m, :], w1_sb[:, 1, ms], xbuf[:, 1, qs],
                                         start=False, stop=True)
                        nc.scalar.activation(gbuf[:, m, qs], gps[:, m, :], AF.Gelu,
                                             bias=b1_sb[:, m:m + 1])
                    for t in range(4):
                        gtile = sb * SB + q * 4 + t
                        cs = slice(q * 512 + t * 128, q * 512 + (t + 1) * 128)
                        att = attps.tile([128, 256], F32, tag="att")
                        for k in range(4):
                            nc.tensor.matmul(att, gbuf[:, k, cs], w2_sb[:, k, :],
                                             start=(k == 0), stop=False)
                        nc.tensor.matmul(att, ones_sb, brow_sb[:, 2, :],
                                         start=False, stop=True)
                        ot = outp.tile([128, 256], F32, tag="out")
                        nc.vector.tensor_add(out=ot, in0=st[("x2", gtile)], in1=att)
                        nc.sync.dma_start(out=out_d[gtile * 128:(gtile + 1) * 128, :], in_=ot)

            def body():
                st.clear()
                for s in range(NSB):
                    xn3T_buf = bigp.tile([128, 2, SB * 128], F32R, tag="xn3T")
                    st[("xn3T", s)] = xn3T_buf
                # phase schedule with 1-tile lag for the neighbor window
                front_done = 0
                mid_done = 0

                def run_front(upto):
                    nonlocal front_done
                    while front_done < upto:
                        front(front_done)
                        front_done += 1

                def run_mid(upto):
                    nonlocal mid_done
                    while mid_done < upto:
                        mid(mid_done)
                        mid_done += 1

                run_front(SB)            # front 0..15
                run_mid(SB - 1)          # mid 0..14
                for s in range(1, NSB):
                    run_front((s + 1) * SB)      # front of next superblock
                    run_mid((s + 1) * SB - 1)    # mid up to boundary-1
                    ffn(s - 1)
                run_mid(NT)              # mid 63 (uses replicate edge)
                ffn(NSB - 1)

            if repeat > 1:
                with tc.For_i(0, repeat, 1):
                    body()
            else:
                body()

    nc.compile()
    return nc


# ---------------------------------------------------------------- entry

def _run(inputs, repeat=1, n_calls=1):
    import time
    consts, corr, need_corr = _host_consts(inputs)
    nc = build_nc(repeat=repeat, need_corr=need_corr)
    x = np.asarray(inputs["x"], np.float32)
    in_maps = []
    for b in range(B):
        m = {"x": np.ascontiguousarray(x[b])}
        for k, v in consts.items():
            m[k] = v
        if need_corr:
            m["corr"] = corr
        in_maps.append(m)
    times = []
    res = None
    for _ in range(n_calls):
        t0 = time.time()
        res = bass_utils.run_bass_kernel_spmd(nc, in_maps, core_ids=list(range(B)))
        times.append(time.time() - t0)
    out = np.stack([res.results[b]["out"] for b in range(B)]).astype(np.float32)
    return out, times


def kernel(**inputs) -> np.ndarray:
    out, _ = _run(inputs, repeat=1, n_calls=1)
    return out


# revision 22
# speedup vs baseline: 88.3474x; 88.3474x over previous
"""Trainium2 Bass kernel for nn_BandProcessor (temporal decay window +
neighbor window + FFN transformer block).

Contract: kernel(**inputs) takes the FULL inputs as numpy arrays and
returns the FULL [8, 8192, 256] float32 output, distributing batch
elements across the 8 NeuronCores (pure data parallel; DxD params
replicated).

Self-contained: hardcodes all shapes; no sibling imports.
"""

import numpy as np
import ml_dtypes

import concourse.bacc as bacc
import concourse.mybir as mybir
from concourse.tile import TileContext
from concourse import bass_utils

B, T, D = 8, 8192, 256
H = 16
DECAY = 0.9
EPS = 1e-5
NT = T // 128          # 64 token tiles per core
SB = 16                # tiles per superblock (FFN batching)
NSB = NT // SB         # 4 superblocks

F32 = mybir.dt.float32
F32R = mybir.dt.float32r
BF16 = mybir.dt.bfloat16

AF = mybir.ActivationFunctionType
ALU = mybir.AluOpType
GELU = AF.Gelu  # swapped to Identity for CoreSim debugging


# ---------------------------------------------------------------- host prep

def _host_consts(inp):
    """Fold LN gains + value/out projections into single matrices (f64)."""
    g1, b1_ = inp["n1_g"].astype(np.float64), inp["n1_b"].astype(np.float64)
    g2, b2_ = inp["n2_g"].astype(np.float64), inp["n2_b"].astype(np.float64)
    g3, b3_ = inp["n3_g"].astype(np.float64), inp["n3_b"].astype(np.float64)
    t_Wv, t_bv = inp["t_Wv"].astype(np.float64), inp["t_bv"].astype(np.float64)
    t_Wo, t_bo = inp["t_Wo"].astype(np.float64), inp["t_bo"].astype(np.float64)
    a_Wv, a_bv = inp["a_Wv"].astype(np.float64), inp["a_bv"].astype(np.float64)
    a_Wo, a_bo = inp["a_Wo"].astype(np.float64), inp["a_bo"].astype(np.float64)
    f_W1, f_b1 = inp["f_W1"].astype(np.float64), inp["f_b1"].astype(np.float64)
    f_W2, f_b2 = inp["f_W2"].astype(np.float64), inp["f_b2"].astype(np.float64)

    WtWo = t_Wv @ t_Wo
    WaWo = a_Wv @ a_Wo
    Wt_eff = (g1[:, None] * WtWo).astype(np.float32)          # [D, D]
    bt_eff = (b1_ @ WtWo + t_bv @ t_Wo + t_bo).astype(np.float32)
    Wa_eff = (g2[:, None] * WaWo).astype(np.float32)
    ba_eff = (b2_ @ WaWo + a_bv @ a_Wo + a_bo).astype(np.float32)
    W1_eff = (g3[:, None] * f_W1).astype(np.float32)          # [D, 2D]
    b1_eff = (b3_ @ f_W1 + f_b1).astype(np.float32)           # [2D]
    W2 = f_W2.astype(np.float32)                              # [2D, D]
    b2 = f_b2.astype(np.float32)

    # temporal weights: reference weights tw[j] apply to h_pad[j:j+T], i.e.
    # lag delta = H-1-j  ->  w_lag[delta] = tw[H-1-delta]
    tw = DECAY ** np.arange(H, dtype=np.float64)
    tw = tw / tw.sum()
    w_lag = tw[::-1].copy()   # w_lag[d] weights h[t-d]

    band1c = np.zeros((128, 128), np.float64)
    for ti in range(128):
        for to in range(ti, min(128, ti + H)):
            band1c[ti, to] = w_lag[to - ti]
    band1p = np.zeros((128, 128), np.float64)
    for p in range(113, 128):
        for to in range(0, p - 112):          # lag = to + 128 - p in [1, 15]
            band1p[p, to] = w_lag[to + 128 - p]
    band2c = np.zeros((128, 128), np.float64)
    for ti in range(128):
        for to in range(max(0, ti - 1), min(128, ti + 2)):
            band2c[ti, to] = 1.0 / 3.0
    e_prev = np.zeros((128, 128), np.float64); e_prev[127, 0] = 1.0 / 3.0
    e_prev0 = np.zeros((128, 128), np.float64); e_prev0[0, 0] = 1.0 / 3.0
    e_next = np.zeros((128, 128), np.float64); e_next[0, 127] = 1.0 / 3.0
    e_next63 = np.zeros((128, 128), np.float64); e_next63[127, 127] = 1.0 / 3.0

    bf = lambda a: a.astype(ml_dtypes.bfloat16)

    # first-tile correction for the temporal zero-pad of the LN bias term
    c_t = np.cumsum(w_lag)[:H - 1]            # c(t) for t = 0..14
    corr = ((c_t - 1.0)[:, None] * (b1_ @ WtWo)[None, :]).astype(np.float32)

    consts = {
        "wt": np.stack([Wt_eff[0:128], Wt_eff[128:256]]),      # [2,128,256]
        "wa": np.stack([Wa_eff[0:128], Wa_eff[128:256]]),
        "w1": np.stack([W1_eff[0:128], W1_eff[128:256]]),      # [2,128,512]
        "w2": np.stack([W2[k * 128:(k + 1) * 128] for k in range(4)]),  # [4,128,256]
        "band1c": bf(band1c), "band1p": bf(band1p),
        "band2c": bf(band2c), "e_prev": bf(e_prev), "e_prev0": bf(e_prev0),
        "e_next": bf(e_next), "e_next63": bf(e_next63),
        "ones_r": np.ones((1, 128), np.float32),
        "brow": np.stack([bt_eff, ba_eff, b2]),                # [3,256]
        "b1col": b1_eff.reshape(4, 128).T.copy(),              # [128,4]
        "ident": np.eye(128, dtype=np.float32),
    }
    need_corr = bool(np.abs(corr).max() > 0)
    return consts, corr, need_corr


# ---------------------------------------------------------------- bass build

def build_nc(repeat=1, need_corr=False):
    nc = bacc.Bacc("TRN2", target_bir_lowering=False, debug=False, num_devices=8)

    x_d = nc.dram_tensor("x", (T, D), F32, kind="ExternalInput")
    out_d = nc.dram_tensor("out", (T, D), F32, kind="ExternalOutput")
    wt_d = nc.dram_tensor("wt", (2, 128, 256), F32R, kind="ExternalInput")
    wa_d = nc.dram_tensor("wa", (2, 128, 256), F32R, kind="ExternalInput")
    w1_d = nc.dram_tensor("w1", (2, 128, 512), F32R, kind="ExternalInput")
    w2_d = nc.dram_tensor("w2", (4, 128, 256), F32R, kind="ExternalInput")
    bands_d = {}
    for nm in ("band1c", "band1p", "band2c", "e_prev", "e_prev0", "e_next", "e_next63"):
        bands_d[nm] = nc.dram_tensor(nm, (128, 128), BF16, kind="ExternalInput")
    ones_d = nc.dram_tensor("ones_r", (1, 128), F32R, kind="ExternalInput")
    brow_d = nc.dram_tensor("brow", (3, 256), F32R, kind="ExternalInput")
    b1c_d = nc.dram_tensor("b1col", (128, 4), F32, kind="ExternalInput")
    id_d = nc.dram_tensor("ident", (128, 128), F32, kind="ExternalInput")
    corr_d = nc.dram_tensor("corr", (15, 256), F32, kind="ExternalInput") if need_corr else None

    with TileContext(nc) as tc:
        import contextlib
        ctx = contextlib.ExitStack()
        with ctx:
            consts = ctx.enter_context(tc.tile_pool(name="consts", bufs=1))
            xpool = ctx.enter_context(tc.tile_pool(name="xpool", bufs=5))
            xn1p = ctx.enter_context(tc.tile_pool(name="xn1p", bufs=5))
            x1p = ctx.enter_context(tc.tile_pool(name="x1p", bufs=20))
            xn2p = ctx.enter_context(tc.tile_pool(name="xn2p", bufs=20))
            x2p = ctx.enter_context(tc.tile_pool(name="x2p", bufs=34))
            xn3p = ctx.enter_context(tc.tile_pool(name="xn3p", bufs=5))
            aggsb = ctx.enter_context(tc.tile_pool(name="aggsb", bufs=5))
            bigp = ctx.enter_context(tc.tile_pool(name="bigp", bufs=2))
            gelup = ctx.enter_context(tc.tile_pool(name="gelup", bufs=1))
            outp = ctx.enter_context(tc.tile_pool(name="outp", bufs=5))
            smalls = ctx.enter_context(tc.tile_pool(name="smalls", bufs=16))
            psA = ctx.enter_context(tc.tile_pool(name="psA", bufs=2, space="PSUM"))
            gargps = ctx.enter_context(tc.tile_pool(name="gargps", bufs=1, space="PSUM"))

            # ---- load constants once
            wt_sb = consts.tile([128, 2, 256], F32R)
            wa_sb = consts.tile([128, 2, 256], F32R)
            w1_sb = consts.tile([128, 2, 512], F32R)
            w2_sb = consts.tile([128, 4, 256], F32R)
            for k in range(2):
                nc.sync.dma_start(out=wt_sb[:, k, :], in_=wt_d[k, :, :])
                nc.sync.dma_start(out=wa_sb[:, k, :], in_=wa_d[k, :, :])
                nc.sync.dma_start(out=w1_sb[:, k, :], in_=w1_d[k, :, :])
            for k in range(4):
                nc.sync.dma_start(out=w2_sb[:, k, :], in_=w2_d[k, :, :])
            band_sb = {}
            for nm, d in bands_d.items():
                tb = consts.tile([128, 128], BF16, tag=nm)
                nc.sync.dma_start(out=tb, in_=d[:, :])
                band_sb[nm] = tb
            ones_sb = consts.tile([1, 128], F32R, tag="ones")
            nc.sync.dma_start(out=ones_sb, in_=ones_d[:, :])
            brow_sb = consts.tile([1, 3, 256], F32R, tag="brow")
            nc.sync.dma_start(out=brow_sb, in_=brow_d[:, :])
            b1_sb = consts.tile([128, 4], F32, tag="b1c")
            nc.sync.dma_start(out=b1_sb, in_=b1c_d[:, :])
            id_sb = consts.tile([128, 128], F32, tag="ident")
            nc.sync.dma_start(out=id_sb, in_=id_d[:, :])
            eps_sb = consts.tile([128, 1], F32, tag="eps")
            nc.vector.memset(eps_sb, EPS)
            corr_sb = None
            if need_corr:
                corr_sb = consts.tile([15, 256], F32, tag="corr")
                nc.sync.dma_start(out=corr_sb, in_=corr_d[:, :])

            # per-iteration state (tile handles keyed by tile index)
            st = {}
            from concourse.tile import add_dep_helper
            tabact_state = {"last": None}

            def chain_tab(inst):
                # serialize table-based ACT ops in emission order so
                # LoadActFuncSet only fires at real phase boundaries
                if tabact_state["last"] is not None:
                    add_dep_helper(inst.ins, tabact_state["last"].ins,
                                   reason="act-table phase ordering")
                tabact_state["last"] = inst

            def ln(src_ap, out_tile):
                s = smalls.tile([128, 6], F32, tag="bnst")
                nc.vector.bn_stats(s, src_ap)
                mv = smalls.tile([128, 2], F32, tag="bnmv")
                nc.vector.bn_aggr(mv, s)
                std = smalls.tile([128, 1], F32, tag="std")
                chain_tab(nc.scalar.activation(std, mv[:, 1:2], AF.Sqrt, bias=eps_sb[:, 0:1]))
                rstd = smalls.tile([128, 1], F32, tag="rstd")
                nc.vector.reciprocal(rstd, std)
                nc.vector.tensor_scalar(out=out_tile, in0=src_ap,
                                        scalar1=mv[:, 0:1], scalar2=rstd,
                                        op0=ALU.subtract, op1=ALU.mult)

            def f1(i):
                xt = xpool.tile([128, 256], F32, tag="x")
                nc.sync.dma_start(out=xt, in_=x_d[i * 128:(i + 1) * 128, :])
                st[("x", i)] = xt
                xn1 = xn1p.tile([128, 256], BF16, tag="xn1")
                ln(xt, xn1)
                st[("xn1", i)] = xn1

            def f2(i):
                xn1 = st[("xn1", i)]
                a1ps = psA.tile([128, 2, 128], F32, tag="aggT")
                for h in range(2):
                    hs = slice(h * 128, (h + 1) * 128)
                    nc.tensor.matmul(a1ps[:, h, :], xn1[:, hs], band_sb["band1c"],
                                     start=True, stop=(i == 0))
                    if i > 0:
                        nc.tensor.matmul(a1ps[:, h, :], st[("xn1", i - 1)][:, hs],
                                         band_sb["band1p"], start=False, stop=True)
                a1sb = aggsb.tile([128, 2, 128], F32R, tag="aggTsb")
                nc.scalar.activation(a1sb, a1ps, AF.Copy)
                st[("a1sb", i)] = a1sb

            def f3a(i):
                a1sb = st.pop(("a1sb", i))
                att = psA.tile([128, 256], F32, tag="att1")
                nc.tensor.matmul(att, a1sb[:, 0, :], wt_sb[:, 0, :], start=True, stop=False)
                nc.tensor.matmul(att, a1sb[:, 1, :], wt_sb[:, 1, :], start=False, stop=False)
                nc.tensor.matmul(att, ones_sb, brow_sb[:, 0, :], start=False, stop=True)
                st[("att1", i)] = att

            def f3b(i):
                att = st.pop(("att1", i))
                x1 = x1p.tile([128, 256], F32, tag="x1")
                nc.vector.tensor_add(out=x1, in0=st.pop(("x", i)), in1=att)
                if need_corr and i == 0:
                    nc.vector.tensor_add(out=x1[0:15, :], in0=x1[0:15, :], in1=corr_sb)
                st[("x1", i)] = x1
                xn2 = xn2p.tile([128, 256], BF16, tag="xn2")
                ln(x1, xn2)
                st[("xn2", i)] = xn2

            def m1(j):
                xn2 = st[("xn2", j)]
                xn2_prev = st[("xn2", j - 1)] if j > 0 else st[("xn2", 0)]
                xn2_next = st[("xn2", j + 1)] if j < NT - 1 else st[("xn2", NT - 1)]
                eprev = band_sb["e_prev"] if j > 0 else band_sb["e_prev0"]
                enext = band_sb["e_next"] if j < NT - 1 else band_sb["e_next63"]
                a2ps = psA.tile([128, 2, 128], F32, tag="aggT")
                for h in range(2):
                    hs = slice(h * 128, (h + 1) * 128)
                    nc.tensor.matmul(a2ps[:, h, :], xn2[:, hs], band_sb["band2c"],
                                     start=True, stop=False)
                    nc.tensor.matmul(a2ps[:, h, :], xn2_prev[:, hs], eprev,
                                     start=False, stop=False)
                    nc.tensor.matmul(a2ps[:, h, :], xn2_next[:, hs], enext,
                                     start=False, stop=True)
                a2sb = aggsb.tile([128, 2, 128], F32R, tag="aggTsb")
                nc.scalar.activation(a2sb, a2ps, AF.Copy)
                st[("a2sb", j)] = a2sb

            def m2a(j):
                a2sb = st.pop(("a2sb", j))
                att = psA.tile([128, 256], F32, tag="att2")
                nc.tensor.matmul(att, a2sb[:, 0, :], wa_sb[:, 0, :], start=True, stop=False)
                nc.tensor.matmul(att, a2sb[:, 1, :], wa_sb[:, 1, :], start=False, stop=False)
                nc.tensor.matmul(att, ones_sb, brow_sb[:, 1, :], start=False, stop=True)
                st[("att2", j)] = att

            def m2b(j):
                att = st.pop(("att2", j))
                x2 = x2p.tile([128, 256], F32, tag="x2")
                nc.vector.tensor_add(out=x2, in0=st.pop(("x1", j)), in1=att)
                st[("x2", j)] = x2
                xn3 = xn3p.tile([128, 256], F32, tag="xn3")
                ln(x2, xn3)
                st[("xn3", j)] = xn3

            def m3(j):
                xn3 = st.pop(("xn3", j))
                x3ps = psA.tile([128, 2, 128], F32, tag="aggT")
                for h in range(2):
                    nc.tensor.transpose(x3ps[:, h, :], xn3[:, h * 128:(h + 1) * 128], id_sb)
                buf = st[("xn3T", j // SB)]
                jr = j % SB
                nc.scalar.activation(buf[:, :, jr * 128:(jr + 1) * 128], x3ps, AF.Copy)

            def ffn_gelu(sb):
                xbuf = st[("xn3T", sb)]
                gbuf = gelup.tile([128, 4, 4 * 512], F32R, tag="geluT")
                st[("gbuf", sb)] = gbuf
                for q in range(4):
                    qs = slice(q * 512, (q + 1) * 512)
                    for m in range(4):
                        gps = gargps.tile([128, 512], F32, tag="garg")
                        ms = slice(m * 128, (m + 1) * 128)
                        nc.tensor.matmul(gps, w1_sb[:, 0, ms], xbuf[:, 0, qs],
                                         start=True, stop=False)
                        nc.tensor.matmul(gps, w1_sb[:, 1, ms], xbuf[:, 1, qs],
                                         start=False, stop=True)
                        chain_tab(nc.scalar.activation(gbuf[:, m, qs], gps, GELU,
                                                       bias=b1_sb[:, m:m + 1]))

            def w2out(gtile):
                sb = gtile // SB
                gbuf = st[("gbuf", sb)]
                r = gtile % SB
                cs = slice(r * 128, (r + 1) * 128)
                att = gargps.tile([128, 256], F32, tag="att3")
                for k in range(4):
                    nc.tensor.matmul(att, gbuf[:, k, cs], w2_sb[:, k, :],
                                     start=(k == 0), stop=False)
                nc.tensor.matmul(att, ones_sb, brow_sb[:, 2, :],
                                 start=False, stop=True)
                ot = outp.tile([128, 256], F32, tag="out")
                nc.vector.tensor_add(out=ot, in0=st.pop(("x2", gtile)), in1=att)
                nc.sync.dma_start(out=out_d[gtile * 128:(gtile + 1) * 128, :], in_=ot)

            # software-pipelined emission: 6-stage skew so every engine
            # always has independent work queued; FFN bursts per superblock
            def body():
                st.clear()
                for s in range(NSB):
                    xn3T_buf = bigp.tile([128, 2, SB * 128], F32R, tag="xn3T")
                    st[("xn3T", s)] = xn3T_buf
                stages = [(f1, 0), (f2, 1), (f3a, 2), (f3b, 3),
                          (m1, 4), (m2a, 5), (m2b, 6), (m3, 7)]
                import collections as _c
                w2q = _c.deque()
                for s_ in range(NT + 8 + 2 * SB):
                    for fn, d_ in stages:
                        i = s_ - d_
                        if 0 <= i < NT:
                            fn(i)
                    jm3 = s_ - 7
                    if 0 <= jm3 < NT and jm3 % SB == SB - 1:
                        ffn_gelu(jm3 // SB)
                        w2q.extend(range(jm3 - SB + 1, jm3 + 1))
                    if w2q:
                        w2out(w2q.popleft())
                while w2q:
                    w2out(w2q.popleft())

            if repeat > 1:
                with tc.For_i(0, repeat, 1):
                    body()
            else:
                body()

    nc.compile()
    return nc


# ---------------------------------------------------------------- entry

def _run(inputs, repeat=1, n_calls=1):
    import time
    consts, corr, need_corr = _host_consts(inputs)
    nc = build_nc(repeat=repeat, need_corr=need_corr)
    x = np.asarray(inputs["x"], np.float32)
    in_maps = []
    for b in range(B):
        m = {"x": np.ascontiguousarray(x[b])}
        for k, v in consts.items():
            m[k] = v
        if need_corr:
            m["corr"] = corr
        in_maps.append(m)
    times = []
    res = None
    for _ in range(n_calls):
        t0 = time.time()
        res = bass_utils.run_bass_kernel_spmd(nc, in_maps, core_ids=list(range(B)))
        times.append(time.time() - t0)
    out = np.stack([res.results[b]["out"] for b in range(B)]).astype(np.float32)
    return out, times


def kernel(**inputs) -> np.ndarray:
    try:
        out, _ = _run(inputs, repeat=1, n_calls=1)
    except Exception:
        # transient device wedges have been observed; one retry
        out, _ = _run(inputs, repeat=1, n_calls=1)
    return out
